# revision 21
# baseline (speedup 1.0000x reference)
"""Deformable conv v1 (dual-branch DFEM) Trainium2 kernel.

Sharding: 8 cores = 4 batches x 2 deformable branches (x/y). Each core
computes, for its (batch, branch):
  offsets = conv3x3(x, pw) + pb           (fp32r matmuls)
  bilinear gather + interp of x at offset positions (dma_gather + DVE)
  y_br    = deform conv3x3 (9 shifted matmuls, bf16)
  part    = w_o[:, branch_half] @ y_br    (1x1 conv half)
Host: out[b] = part_x + part_y + x[b] + b_o.

Data layouts (per core):
  pos = c*128 + p  (c: 32 chunks, p: partition in pos-space tensors)
  kl (within a 1024-pos gather call, chunk group c4) = c8*128 + p, c = c4*8+c8
  gather idx wrap: idxs[pw, s] = idx(kl = s*16 + pw)  (hardware unwrap order)
  gather elem (X2 row r*66+y): [y0|r0|256c, y0|r1, y1|r0, y1|r1] -> out
  block e = y*4 + r*2 + chb, partition = ch%128, col = kl.
"""

import numpy as np
import ml_dtypes

import concourse.bacc as bacc
import concourse.bass as bass
import concourse.mybir as mybir
from concourse import tile
from concourse.bass_types import AP
from concourse.bass_utils import run_bass_kernel_spmd
from concourse.library_config import mlp
from concourse.masks import make_identity

F32 = mybir.dt.float32
F32R = mybir.dt.float32r
BF16 = mybir.dt.bfloat16
I16 = mybir.dt.int16
AL = mybir.AluOpType

C = 256
H = W = 64
HP = WP = 66
NPOS = H * W          # 4096
NK = 9                # kernel points
NCHUNK = 32           # pos chunks of 128
NF = NCHUNK * NK      # 288
X2_ROWS = 68 * 66     # 4488
ELEM = 1024           # values per gather element (2y x 2r x 256c) bf16
ESTEP = 512           # element stride in values
CALL = 1024           # idxs per gather call
NC4 = 4               # pos chunk-groups of 1024


def _ts(v, out, in0, s1, s2, op0, op1=AL.bypass):
    v.tensor_scalar(out, in0, s1, s2, op0, op1)


def deform_kernel(tc, nc, x2, xp, pwt, wmt, wot, cbx, cby, out, dbg):
    v = nc.vector
    sc = nc.scalar

    with tc.tile_pool(name="consts", bufs=1) as consts, \
         tc.tile_pool(name="persist", bufs=1) as pers:
        ident = consts.tile([128, 128], F32)
        make_identity(nc, ident)
        pwt_sb = consts.tile([128, NK * 2 * 18], F32R)
        nc.sync.dma_start(pwt_sb[:], pwt.ap())
        cb_sb = consts.tile([128, 2 * NF], F32)
        nc.sync.dma_start(cb_sb[:, 0:NF], cbx.ap())
        nc.sync.dma_start(cb_sb[:, NF:], cby.ap())
        oh = consts.tile([32, 32 * 128], BF16)
        nc.gpsimd.memset(oh[:], 0.0)
        # oh[p, cc*128+q] = (p == cc): affine val = p - cc; != 0 -> keep 0
        nc.gpsimd.affine_select(
            out=oh[:].rearrange("p (cc q) -> p cc q", cc=32),
            in_=oh[:].rearrange("p (cc q) -> p cc q", cc=32),
            compare_op=AL.not_equal, fill=1.0, base=0,
            pattern=[[-1, 32], [0, 128]], channel_multiplier=1)
        wmt_sb = consts.tile([128, NK * 4 * 128], BF16)
        nc.sync.dma_start(wmt_sb[:], wmt.ap())
        wot_sb = consts.tile([128, 4 * 128], BF16)
        nc.sync.dma_start(wot_sb[:], wot.ap())

        # persistent intermediates
        wrap = pers.tile([128, NK * 256], I16)    # [pw(+16k),(n c4 c8 ph)]
        wT4 = pers.tile([32, NK * 4 * 128], BF16)  # [c,(n w p)]
        y_sb = [pers.tile([128, NPOS], BF16, tag=f"y{k}", name=f"y{k}")
                for k in range(2)]

        # ============ Phase 1+2: offset conv, idx/weight pipeline =========
        with tc.tile_pool(name="xp", bufs=1) as xp_p, \
             tc.tile_pool(name="off", bufs=1) as off_p, \
             tc.tile_pool(name="pipe", bufs=1) as pipe, \
             tc.tile_pool(name="stage", bufs=2) as stage_p, \
             tc.tile_pool(name="psum_off", bufs=2, space="PSUM") as psum_off, \
             tc.tile_pool(name="psum_tr", bufs=2, space="PSUM") as psum_tr:
            xp_sb = [xp_p.tile([128, HP * WP], F32R, tag=f"xp{k}", name=f"xp{k}")
                     for k in range(2)]
            for k in range(2):
                nc.sync.dma_start(xp_sb[k][:], xp.ap()[k])

            off_sb = off_p.tile([18, NPOS], F32)
            pwt3 = pwt_sb[:].rearrange("p (s k m) -> p s k m", s=NK, k=2)
            for nt in range(8):
                ps = psum_off.tile([18, 512], F32, tag="po")
                first = True
                for s in range(NK):
                    kh, kw = s // 3, s % 3
                    for k in range(2):
                        rhs = xp_sb[k][:].rearrange("p (r w) -> p r w", w=WP)[
                            :, nt * 8 + kh: nt * 8 + kh + 8, kw: kw + W]
                        nc.tensor.matmul(
                            ps[:], pwt3[:, s, k, :],
                            rhs,
                            start=first, stop=(s == NK - 1 and k == 1))
                        first = False
                sc.copy(off_sb[:, nt * 512:(nt + 1) * 512], ps[:])
            if dbg:
                nc.sync.dma_start(dbg["off"].ap(), off_sb[:])

            # transpose offsets: offT[p, c, ch] = off[ch, c*128+p]
            offT = pipe.tile([128, NCHUNK * 18], F32)
            offT3 = offT[:].rearrange("p (c h) -> p c h", h=18)
            for c in range(NCHUNK):
                pt = psum_tr.tile([128, 18], F32, tag="pt")
                nc.tensor.transpose(pt[:], off_sb[:, c * 128:(c + 1) * 128],
                                    ident[0:18, 0:18])
                sc.copy(offT3[:, c, :], pt[:])

            # ---- index & weight arithmetic (x in cols 0:NF, y in NF:2NF)
            t_px = pipe.tile([128, 2 * NF], F32)
            t_q = pipe.tile([128, 2 * NF], F32)
            t_a = pipe.tile([128, 2 * NF], F32)
            t_b = pipe.tile([128, 2 * NF], F32)
            t_t0 = pipe.tile([128, 2 * NF], F32)
            t_t1 = pipe.tile([128, 2 * NF], F32)

            for ax in range(2):
                sl = slice(ax * NF, (ax + 1) * NF)
                v.tensor_add(
                    t_px[:, sl].rearrange("p (c n) -> p c n", n=NK),
                    offT3[:, :, ax * NK:(ax + 1) * NK],
                    cb_sb[:, sl].rearrange("p (c n) -> p c n", n=NK))
            # floor(px): int-cast (any rounding) then correct where y > px
            i32 = pipe.tile([128, 2 * NF], mybir.dt.int32)
            v.tensor_copy(i32[:], t_px[:])
            v.tensor_copy(t_t1[:], i32[:])
            v.tensor_tensor(t_t0[:], t_t1[:], t_px[:], AL.is_gt)
            v.tensor_sub(t_t1[:], t_t1[:], t_t0[:])              # floor(px)
            _ts(v, t_q[:], t_t1[:], 0.0, float(HP - 1), AL.max, AL.min)
            _ts(v, t_t0[:], t_t1[:], 1.0, 0.0, AL.add, AL.max)
            _ts(v, t_t0[:], t_t0[:], float(HP - 1), None, AL.min)  # qrb
            _ts(v, t_px[:], t_px[:], 0.0, float(HP - 1), AL.max, AL.min)
            v.tensor_sub(t_b[:], t_px[:], t_q[:])                # b = pc-qlt
            v.tensor_sub(t_t0[:], t_t0[:], t_q[:])
            v.tensor_add(t_t0[:], t_t0[:], t_b[:])
            _ts(v, t_a[:], t_t0[:], -1.0, 2.0, AL.mult, AL.add)  # a

            _ts(v, t_t0[:, 0:NF], t_q[:, 0:NF], float(WP), None, AL.mult)
            v.tensor_add(t_t0[:, 0:NF], t_t0[:, 0:NF], t_q[:, NF:])
            idx16 = pipe.tile([128, NF], I16)
            v.tensor_copy(idx16[:], t_t0[:, 0:NF])

            w4 = pipe.tile([128, 4 * NF], F32)
            v.tensor_mul(w4[:, 0 * NF:1 * NF], t_a[:, 0:NF], t_a[:, NF:])
            v.tensor_mul(w4[:, 1 * NF:2 * NF], t_b[:, 0:NF], t_a[:, NF:])
            v.tensor_mul(w4[:, 2 * NF:3 * NF], t_a[:, 0:NF], t_b[:, NF:])
            v.tensor_mul(w4[:, 3 * NF:4 * NF], t_b[:, 0:NF], t_b[:, NF:])
            w4b = pipe.tile([128, 4 * NF], BF16)
            v.tensor_copy(w4b[:], w4[:])
            if dbg:
                nc.sync.dma_start(dbg["w4"].ap(), w4[:])

            # ---- stream-transpose cascade
            idxT = pipe.tile([32, NK * 128 + 32], I16)  # [c, n*128+p] (+pad)
            v.memset(idxT[:, NK * 128:], 0)
            idx3 = idx16[:].rearrange("p (c n) -> p c n", n=NK)
            w4b4 = w4b[:].rearrange("p (w c n) -> p w c n", w=4, n=NK)
            for n in range(NK):
                for pb in range(4):
                    v.transpose(
                        idxT[:, n * 128 + pb * 32: n * 128 + (pb + 1) * 32],
                        idx3[pb * 32:(pb + 1) * 32, :, n])
                    for w in range(4):
                        v.transpose(
                            wT4[:, (n * 4 + w) * 128 + pb * 32:
                                (n * 4 + w) * 128 + (pb + 1) * 32],
                            w4b4[pb * 32:(pb + 1) * 32, w, :, n])

            wrap5 = wrap[:].rearrange("p (n f c8 ph) -> p n f c8 ph",
                                      n=NK, f=NC4, c8=8)
            for n in range(NK):
                for ph in range(8):  # overlapping 16-col-shifted windows
                    st = stage_p.tile([32, 32], I16, tag="st")
                    v.transpose(st[:], idxT[:, n * 128 + ph * 16:
                                            n * 128 + ph * 16 + 32])
                    src = st[0:16, :].rearrange("p (f c8) -> p f c8", f=NC4)
                    v.tensor_copy(wrap5[0:16, n, :, :, ph], src)
            for k in range(1, 8):
                nc.sync.dma_start(wrap[16 * k:16 * (k + 1), :], wrap[0:16, :])
            if dbg:
                nc.sync.dma_start(dbg["idx"].ap(), wrap[:])

        # ============ Phase 3: gather + interp + main conv ================
        nc.gpsimd.load_library(mlp)
        x2flat = x2.ap().rearrange("r e -> (r e)")
        x2gap = AP(x2flat.tensor, x2flat.offset,
                   [[ESTEP, X2_ROWS - 1], [1, ELEM]])

        with tc.tile_pool(name="gather", bufs=3) as g_p, \
             tc.tile_pool(name="xoff", bufs=3) as xo_p, \
             tc.tile_pool(name="w4s", bufs=2) as w4s_p, \
             tc.tile_pool(name="psum_rep", bufs=2, space="PSUM") as psum_rep, \
             tc.tile_pool(name="psum_main", bufs=1, space="PSUM") as psum_main:
            wm4 = wmt_sb[:].rearrange("p (n k m q) -> p n k m q", n=NK, k=2, m=2)
            for c4 in range(NC4):
                psm = [psum_main.tile([128, 512], F32, tag=f"pm{i}", name=f"pm{i}")
                       for i in range(4)]
                for n in range(NK):
                    # replicate weights -> W4s[:, w*1024 + c8*128 + p]
                    w4s = w4s_p.tile([128, 4 * CALL], BF16, tag="w4s")
                    for half in range(4):
                        pr = psum_rep.tile([128, 1024], F32, tag="pr")
                        for c8h in range(2):
                            cc = c4 * 8 + half * 2 + c8h
                            nc.tensor.matmul(
                                pr[:, c8h * 512:(c8h + 1) * 512],
                                oh[:, cc * 128:(cc + 1) * 128],
                                wT4[:, n * 512:(n + 1) * 512],
                                start=True, stop=True)
                        src = pr[:].rearrange("p (c w q) -> p c w q", c=2, w=4)
                        dst = w4s[:].rearrange(
                            "p (w c q) -> p c w q", w=4, c=8)[
                            :, half * 2:(half + 1) * 2, :, :]
                        sc.copy(dst, src)

                    g = g_p.tile([128, 8 * CALL], BF16, tag="g")
                    g3 = g[:].rearrange("p (e t) -> p e t", e=8)
                    idxs = wrap[:, (n * NC4 + c4) * 64:(n * NC4 + c4 + 1) * 64]
                    nc.gpsimd.dma_gather(g3, x2gap, idxs, CALL, CALL, ELEM,
                                         elem_step=ESTEP, transpose=True,
                                         single_packet=False)

                    # pair-merged interp: both chalf blocks per TT (FD 2048)
                    def wrep(w):
                        base = w4s[:, w * CALL:(w + 1) * CALL]
                        return AP(base.tensor, base.offset,
                                  [base.ap[0], [0, 2], [1, CALL]])

                    xo = xo_p.tile([128, 2 * CALL], BF16, tag="xo")
                    tmp = xo_p.tile([128, 2 * CALL], BF16, tag="tmp")
                    xo3 = xo[:].rearrange("p (u t) -> p u t", u=2)
                    tmp3 = tmp[:].rearrange("p (u t) -> p u t", u=2)
                    v.tensor_mul(xo3, g3[:, 0:2, :], wrep(0))
                    v.tensor_mul(tmp3, g3[:, 2:4, :], wrep(1))
                    v.tensor_add(xo[:], xo[:], tmp[:])
                    v.tensor_mul(tmp3, g3[:, 4:6, :], wrep(2))
                    v.tensor_add(xo[:], xo[:], tmp[:])
                    v.tensor_mul(tmp3, g3[:, 6:8, :], wrep(3))
                    v.tensor_add(xo[:], xo[:], tmp[:])
                    for h in range(2):
                        if dbg:
                            nc.sync.dma_start(
                                dbg["xoff"].ap()[h][
                                    :, n * NPOS + c4 * CALL:
                                    n * NPOS + (c4 + 1) * CALL],
                                xo[:, h * CALL:(h + 1) * CALL])
                        for mt in range(2):
                            for nt2 in range(2):
                                nc.tensor.matmul(
                                    psm[mt * 2 + nt2][:],
                                    wm4[:, n, h, mt, :],
                                    xo[:, h * CALL + nt2 * 512:
                                       h * CALL + (nt2 + 1) * 512],
                                    start=(n == 0 and h == 0),
                                    stop=(n == NK - 1 and h == 1))
                for mt in range(2):
                    for nt2 in range(2):
                        sc.copy(y_sb[mt][:, c4 * CALL + nt2 * 512:
                                         c4 * CALL + (nt2 + 1) * 512],
                                psm[mt * 2 + nt2][:])
        if dbg:
            for k in range(2):
                nc.sync.dma_start(dbg["ybr"].ap()[k], y_sb[k][:])

        # ============ Phase 4: 1x1 conv ===================================
        with tc.tile_pool(name="outp", bufs=1) as out_p, \
             tc.tile_pool(name="psum_o", bufs=4, space="PSUM") as psum_o:
            wo3 = wot_sb[:].rearrange("p (k m q) -> p k m q", k=2, m=2)
            for mt in range(2):
                out_sb = out_p.tile([128, NPOS], F32, tag=f"o{mt}")
                for nt in range(8):
                    po = psum_o.tile([128, 512], F32, tag="pso")
                    for kt in range(2):
                        nc.tensor.matmul(po[:], wo3[:, kt, mt, :],
                                         y_sb[kt][:, nt * 512:(nt + 1) * 512],
                                         start=(kt == 0), stop=(kt == 1))
                    sc.copy(out_sb[:, nt * 512:(nt + 1) * 512], po[:])
                nc.sync.dma_start(out.ap()[mt], out_sb[:])


def build_program(debug=False):
    nc = bacc.Bacc("TRN2", target_bir_lowering=False, debug=False)
    x2 = nc.dram_tensor("x2", [X2_ROWS, ESTEP], BF16, kind="ExternalInput")
    xp = nc.dram_tensor("xp", [2, 128, HP * WP], F32R, kind="ExternalInput")
    pwt = nc.dram_tensor("pwt", [128, NK * 2 * 18], F32R, kind="ExternalInput")
    wmt = nc.dram_tensor("wmt", [128, NK * 4 * 128], BF16, kind="ExternalInput")
    wot = nc.dram_tensor("wot", [128, 4 * 128], BF16, kind="ExternalInput")
    cbx = nc.dram_tensor("cbx", [128, NF], F32, kind="ExternalInput")
    cby = nc.dram_tensor("cby", [128, NF], F32, kind="ExternalInput")
    out = nc.dram_tensor("out", [2, 128, NPOS], F32, kind="ExternalOutput")
    dbg = {}
    if debug:
        dbg["off"] = nc.dram_tensor("dbg_off", [18, NPOS], F32,
                                    kind="ExternalOutput")
        dbg["idx"] = nc.dram_tensor("dbg_idx", [128, NK * 256], I16,
                                    kind="ExternalOutput")
        dbg["w4"] = nc.dram_tensor("dbg_w4", [128, 4 * NF], F32,
                                   kind="ExternalOutput")
        dbg["xoff"] = nc.dram_tensor("dbg_xoff", [2, 128, NK * NPOS], BF16,
                                     kind="ExternalOutput")
        dbg["ybr"] = nc.dram_tensor("dbg_ybr", [2, 128, NPOS], BF16,
                                    kind="ExternalOutput")

    with tile.TileContext(nc) as tc:
        deform_kernel(tc, nc, x2, xp, pwt, wmt, wot, cbx, cby, out, dbg)
    nc.compile()
    return nc


# ======================= host side =======================================

_cached = {}


def _prep_core(xb, pw, pb, w, w_o, beta):
    """Inputs for one core (batch image xb (256,64,64), branch beta)."""
    xpad = np.zeros((C, HP, WP), np.float32)
    xpad[:, 1:65, 1:65] = xb
    xt = xpad.transpose(1, 2, 0)  # (66,66,256)
    x2v = np.zeros((68, WP, 2 * C), np.float32)
    x2v[0:66, :, 0:C] = xt
    x2v[0:65, :, C:] = xt[1:66]
    x2 = x2v.reshape(X2_ROWS, ESTEP).astype(ml_dtypes.bfloat16)
    xp = xpad.reshape(2, 128, HP * WP).astype(np.float32)
    pwt = pw.reshape(18, 2, 128, 3, 3).transpose(2, 3, 4, 1, 0).reshape(
        128, NK * 2 * 18).astype(np.float32)
    wmt = w.reshape(2, 128, 2, 128, 3, 3).transpose(3, 4, 5, 2, 0, 1).reshape(
        128, NK * 4 * 128).astype(ml_dtypes.bfloat16)
    half = w_o[:, beta * C:(beta + 1) * C, 0, 0]
    wot = half.reshape(2, 128, 2, 128).transpose(3, 2, 0, 1).reshape(
        128, 4 * 128).astype(ml_dtypes.bfloat16)
    p = np.arange(128)[:, None, None]
    c = np.arange(NCHUNK)[None, :, None]
    n = np.arange(NK)[None, None, :]
    pos = c * 128 + p
    cbx = (pos // W + n // 3 + pb[None, None, 0:NK]).astype(
        np.float32).reshape(128, NF)
    cby = (pos % W + n % 3 + pb[None, None, NK:]).astype(
        np.float32).reshape(128, NF)
    return {"x2": np.ascontiguousarray(x2), "xp": np.ascontiguousarray(xp),
            "pwt": np.ascontiguousarray(pwt),
            "wmt": np.ascontiguousarray(wmt),
            "wot": np.ascontiguousarray(wot), "cbx": cbx, "cby": cby}


def kernel(x, pw_x, pb_x, w_x, pw_y, pb_y, w_y, w_o, b_o):
    x = np.asarray(x, np.float32)
    if "nc" not in _cached:
        _cached["nc"] = build_program()
    nc = _cached["nc"]
    B = x.shape[0]
    in_maps = []
    for b in range(B):
        for pw, pb, w, beta in [(pw_x, pb_x, w_x, 0), (pw_y, pb_y, w_y, 1)]:
            in_maps.append(_prep_core(x[b], np.asarray(pw, np.float32),
                                      np.asarray(pb, np.float32),
                                      np.asarray(w, np.float32),
                                      np.asarray(w_o, np.float32), beta))
    import time as _time
    _t0 = _time.time()
    res = run_bass_kernel_spmd(nc, in_maps, core_ids=list(range(2 * B)))
    _cached["exec_wall"] = _time.time() - _t0
    _cached["last_res"] = res
    outs = [r["out"].reshape(C, H, W).astype(np.float32)
            for r in res.results]
    result = np.empty((B, C, H, W), np.float32)
    b_o = np.asarray(b_o, np.float32)
    for b in range(B):
        result[b] = outs[2 * b] + outs[2 * b + 1] + x[b] + b_o[:, None, None]
    return result
